# revision 6
# baseline (speedup 1.0000x reference)
"""Self-contained Trainium2 Bass kernel for a 12-head attention layer.

Problem: x[4,2048,768] -> attention(QKV projections, softmax, context),
NUM_HEADS=12, SIZE_PER_HEAD=64, additive mask from mask[4,2048].

Sharding over 8 NeuronCores: core c handles batch b=c//2 and head-group
hg=c%2 (6 heads, 384 feature columns).  Everything is local per core:
no collectives.

Design (v2):
  - Projections Q^T,K^T [384,2048] and V^T feature-major, all bf16 matmuls
    at full K=128/M=128 utilization; V^T is DMA-XBAR-transposed to a
    token-major vp [128, 16T, 6x(64+1)] layout (65th col = ones, memset)
    so the context matmul's 65th row yields the softmax denominator.
  - Scores: fp8(e4m3) DoubleRow matmuls (2 contraction values per PE
    cell: lhsT/rhs [32,2,*]) -> 2x PE throughput vs bf16 at K=64.  The
    1/sqrt(64) scale is folded into the exp activation's scale, so Q/K
    are quantized at natural scale.  Rel-err budget ~1.3% < 2e-2 gate.
  - Softmax: exp on ACT (the hard bottleneck: 192 x [128,1024] tiles
    ~206us); mask enters as per-partition bias (all-ones mask -> bias 0).
  - Single-head segments (6 heads x 2 F-blocks x 16 T-tiles), scores
    psum double-buffered, ctx psum single [65,1024] + copy-drain;
    projections interleaved as fill so ACT never starves.
  - Normalize per segment: DMA-gather denominators -> DVE reciprocal ->
    DMA scatter -> GpSimd partition_broadcast -> DVE multiply -> DMA out.

Output per core: ctx^T [384,2048] f32; host transposes to [4,2048,768].
"""

import numpy as np
import ml_dtypes

B, S, D = 4, 2048, 768
H, DH = 12, 64
HL = 6          # heads per core
DL = HL * DH    # 384 feature columns per core
NCORES = 8
P = 128
KO = 6          # full k-subtiles of the 768 contraction
NT = S // P     # 16 T-tiles
FB = 1024       # F block size
NFB = S // FB   # 2 F blocks

SCORES_FP8 = True
# token layout inside a T-tile, set by the V transpose DMA semantics:
# True: vp[p, mt, :] holds token p*NT + mt  (XBAR row-major order)
# False: vp[p, mt, :] holds token mt*P + p  (natural; probe-verified)
TOK_INTERLEAVED = False

FP8 = ml_dtypes.float8_e4m3

_CACHE = {}


def _build(with_bias=False, ones_mask=True):
    import concourse.mybir as mybir
    import concourse.tile as tile
    from concourse import bacc

    dt = mybir.dt
    Exp = mybir.ActivationFunctionType.Exp
    DR = mybir.MatmulPerfMode.DoubleRow

    nc = bacc.Bacc("TRN2", target_bir_lowering=False, debug=False,
                   num_devices=NCORES)

    xT = nc.dram_tensor("xT", [D, S], dt.bfloat16, kind="ExternalInput")
    wq = nc.dram_tensor("wq", [D + 1, DL], dt.bfloat16, kind="ExternalInput")
    wk = nc.dram_tensor("wk", [D + 1, DL], dt.bfloat16, kind="ExternalInput")
    wv = nc.dram_tensor("wv", [D + 1, DL], dt.bfloat16, kind="ExternalInput")
    adder = nc.dram_tensor("adder", [P, NT], dt.float32, kind="ExternalInput")
    out = nc.dram_tensor("out", [DL, S], dt.float32, kind="ExternalOutput")

    KE = KO + 1 if with_bias else KO

    with tile.TileContext(nc) as tc:
        with (
            tc.tile_pool(name="persist", bufs=1) as sb,
            tc.tile_pool(name="etp", bufs=10) as etp,
            tc.tile_pool(name="stage", bufs=3) as stg,
            tc.tile_pool(name="vfp", bufs=2) as vfp,
            tc.tile_pool(name="fin", bufs=2) as fin,
            tc.tile_pool(name="ps_s", bufs=2, space="PSUM") as ps_s,
            tc.tile_pool(name="ps_c", bufs=1, space="PSUM") as ps_c,
            tc.tile_pool(name="ps_p", bufs=2, space="PSUM") as ps_p,
        ):
            # ---------------- input DMA (priority order) ----------------
            wqs = sb.tile([P, KO + 1, DL], dt.bfloat16, tag="wqs")
            wks = sb.tile([P, KO + 1, DL], dt.bfloat16, tag="wks")
            wvs = sb.tile([P, KO + 1, DL], dt.bfloat16, tag="wvs")
            xTs = sb.tile([P, KO + 1, S], dt.bfloat16, tag="xTs")
            adder_sb = sb.tile([P, NT], dt.float32, tag="adder")

            def dma_w(w_dram, w_sb, c0, c1):
                nc.sync.dma_start(
                    w_sb[:, 0:KO, c0:c1],
                    w_dram.ap()[0:D, c0:c1].rearrange(
                        "(ko p) m -> p ko m", p=P))
                if with_bias:
                    nc.sync.dma_start(w_sb[0:1, KO, c0:c1],
                                      w_dram.ap()[D:D + 1, c0:c1])

            def dma_x(s0, s1):
                nc.sync.dma_start(
                    xTs[:, 0:KO, s0:s1],
                    xT.ap()[:, s0:s1].rearrange("(ko p) s -> p ko s", p=P))

            dma_w(wq, wqs, 0, P)
            dma_w(wk, wks, 0, P)
            dma_x(0, FB)
            dma_w(wv, wvs, 0, DL)
            dma_x(FB, S)
            dma_w(wq, wqs, P, DL)
            dma_w(wk, wks, P, DL)
            if not ones_mask:
                nc.sync.dma_start(adder_sb[:], adder.ap())
            if with_bias:
                nc.gpsimd.memset(xTs[0:1, KO, :], 1.0)

            # persistent projection outputs
            if SCORES_FP8:
                qt8 = sb.tile([32, HL, 2, S], dt.float8e4, tag="qt8")
                kt8 = sb.tile([32, HL, 2, S], dt.float8e4, tag="kt8")
            else:
                qtb = sb.tile([P, 3, S], dt.bfloat16, tag="qtb")
                ktb = sb.tile([P, 3, S], dt.bfloat16, tag="ktb")
            vp = sb.tile([P, NT, HL * (DH + 1)], dt.bfloat16, tag="vp")
            for h in range(HL):
                nc.gpsimd.memset(vp[:, :, h * (DH + 1) + DH:
                                    h * (DH + 1) + DH + 1], 1.0)

            # ---------------- projection helpers ----------------
            # one 512-wide proj chunk: psum <- W_mtile^T @ x_chunk
            def proj_mm(w_sb, mt, c):
                pt = ps_p.tile([P, 512], dt.float32, tag="p", name="pt")
                for k in range(KE):
                    lhsT = (w_sb[:, k, mt * P:(mt + 1) * P] if k < KO
                            else w_sb[0:1, k, mt * P:(mt + 1) * P])
                    rhs = (xTs[:, k, c * 512:(c + 1) * 512] if k < KO
                           else xTs[0:1, k, c * 512:(c + 1) * 512])
                    nc.tensor.matmul(pt[:], lhsT, rhs,
                                     start=(k == 0), stop=(k == KE - 1))
                return pt

            qk_stage = {}

            def qk_chunk(which, mt, c):
                # proj + fp8 cast; on odd c, fold-DMA the completed S-half
                # into the DoubleRow layout [32, h, j, S].
                w_sb = wqs if which == "q" else wks
                pt = proj_mm(w_sb, mt, c)
                if SCORES_FP8:
                    key = (which, mt, c // 2)
                    if c % 2 == 0:
                        qk_stage[key] = stg.tile([P, FB], dt.float8e4,
                                                 tag="stg", name="stg")
                    st = qk_stage[key]
                    nc.vector.tensor_copy(st[:, (c % 2) * 512:
                                             (c % 2) * 512 + 512], pt[:])
                    if c % 2 == 1:
                        dstt = qt8 if which == "q" else kt8
                        half = c // 2
                        for hip in range(2):
                            h = 2 * mt + hip
                            for j in range(2):
                                nc.sync.dma_start(
                                    dstt[0:32, h, j,
                                         half * FB:(half + 1) * FB],
                                    st[hip * 64 + 32 * j:
                                       hip * 64 + 32 * j + 32, :])
                        del qk_stage[key]
                else:
                    dstt = qtb if which == "q" else ktb
                    nc.vector.tensor_copy(
                        dstt[:, mt, c * 512:(c + 1) * 512], pt[:])

            vf_stage = {}

            def vf_chunk(mt, c):
                # feature-major V' proj chunk; after chunk 3, XBAR-transpose
                # both heads of this m-tile into token-major vp.
                pt = proj_mm(wvs, mt, c)
                if c == 0:
                    vf_stage[mt] = vfp.tile([P, S], dt.bfloat16,
                                            tag="vf", name="vf")
                vt = vf_stage[mt]
                nc.vector.tensor_copy(vt[:, c * 512:(c + 1) * 512], pt[:])
                if c == 3:
                    for hip in range(2):
                        h = 2 * mt + hip
                        # XBAR transpose needs a contiguous destination
                        # (strided dst slices produce wrong output on HW);
                        # bounce through a scratch tile, GpSimd copies into
                        # the strided vp layout.
                        vph = vfp.tile([P, NT, DH], dt.bfloat16,
                                       tag="vph", name="vph")
                        nc.sync.dma_start_transpose(
                            vph[:], vt[hip * DH:(hip + 1) * DH, :])
                        nc.gpsimd.tensor_copy(
                            vp[:, :, h * (DH + 1):h * (DH + 1) + DH],
                            vph[:])
                    del vf_stage[mt]

            # ---------------- warmup (HAM un-throttle during DMA) --------
            warm = sb.tile([P, 512], dt.bfloat16, tag="warm")
            nc.gpsimd.memset(warm[:], 0.0)
            wexp = sb.tile([P, 1], dt.bfloat16, tag="wexp")
            nc.scalar.activation(wexp[:], warm[:, 0:1], Exp)
            wpt = ps_s.tile([P, 512], dt.float32, tag="s", name="wpt")
            for wi in range(26):
                nc.tensor.matmul(wpt[:], warm[:, 0:P], warm[:],
                                 start=(wi == 0), stop=(wi == 25))

            # ---------------- prefix projections ----------------
            qk_chunk("q", 0, 0)
            qk_chunk("q", 0, 1)
            qk_chunk("k", 0, 0)
            qk_chunk("k", 0, 1)

            # ---------------- fill queue ----------------
            from collections import deque
            fills = deque()
            fills.append(lambda: vf_chunk(0, 0))
            fills.append(lambda: vf_chunk(0, 1))
            fills.append(lambda: qk_chunk("k", 0, 2))
            fills.append(lambda: qk_chunk("k", 0, 3))
            fills.append(lambda: vf_chunk(0, 2))
            fills.append(lambda: vf_chunk(0, 3))
            for mt in (1, 2):
                for c in range(4):
                    fills.append(lambda m=mt, cc=c: qk_chunk("q", m, cc))
                for c in range(4):
                    fills.append(lambda m=mt, cc=c: qk_chunk("k", m, cc))
                for c in range(4):
                    fills.append(lambda m=mt, cc=c: vf_chunk(m, cc))
            # q0c23 (needed by seg 2 = (h0,fb1)) goes right after seg0's
            # must-haves; m1/m2 chunks follow FIFO.
            fills.insert(6, lambda: qk_chunk("q", 0, 2))
            fills.insert(7, lambda: qk_chunk("q", 0, 3))

            def pop_fill(n):
                for _ in range(n):
                    if fills:
                        fills.popleft()()

            # ---------------- attention segments ----------------
            def emit_scores(h, fb, ti):
                s_ps = ps_s.tile([P, FB], dt.float32, tag="s", name="s_ps")
                if SCORES_FP8:
                    if TOK_INTERLEAVED:
                        ksel = kt8[0:32, h, :, :].rearrange(
                            "p j (t g) -> p j g t", g=NT)[:, :, ti, :]
                    else:
                        ksel = kt8[0:32, h, :, ti * P:(ti + 1) * P]
                    for n in range(2):
                        fc = fb * FB + n * 512
                        nc.tensor.matmul(
                            s_ps[:, n * 512:(n + 1) * 512], ksel,
                            qt8[0:32, h, :, fc:fc + 512],
                            start=True, stop=True, perf_mode=DR)
                else:
                    off = (h % 2) * DH
                    if TOK_INTERLEAVED:
                        ksel = ktb[off:off + DH, h // 2, :].rearrange(
                            "d (t g) -> d g t", g=NT)[:, ti, :]
                    else:
                        ksel = ktb[off:off + DH, h // 2,
                                   ti * P:(ti + 1) * P]
                    for n in range(2):
                        fc = fb * FB + n * 512
                        nc.tensor.matmul(
                            s_ps[:, n * 512:(n + 1) * 512], ksel,
                            qtb[off:off + DH, h // 2, fc:fc + 512],
                            start=True, stop=True)
                return s_ps

            def emit_exp(s_ps, ti):
                et = etp.tile([P, FB], dt.bfloat16, tag="et", name="et")
                bias = 0.0 if ones_mask else adder_sb[:, ti:ti + 1]
                nc.scalar.activation(et[:], s_ps[:], Exp,
                                     bias=bias, scale=0.125)
                return et

            def emit_ctx(ctx_ps, h, ti, et):
                for n in range(2):
                    nc.tensor.matmul(
                        ctx_ps[:, n * 512:(n + 1) * 512],
                        vp[:, ti, h * (DH + 1):(h + 1) * (DH + 1)],
                        et[:, n * 512:(n + 1) * 512],
                        start=(ti == 0), stop=(ti == NT - 1))

            def normalize(h, fb, ctxs_t):
                nfb = FB // P   # 8 denominator cols
                den = fin.tile([P, nfb], dt.float32, tag="den", name="den")
                nc.sync.dma_start(den[:], ctxs_t[DH:DH + 1, :])
                rec = fin.tile([P, nfb], dt.float32, tag="rec", name="rec")
                nc.vector.reciprocal(rec[:], den[:])
                rr = fin.tile([1, FB], dt.float32, tag="rr", name="rr")
                nc.sync.dma_start(rr[:], rec[:])
                rrb = fin.tile([DH, FB], dt.float32, tag="rrb", name="rrb")
                nc.gpsimd.partition_broadcast(rrb[:], rr[:])
                ot = fin.tile([DH, FB], dt.float32, tag="ot", name="ot")
                nc.vector.tensor_tensor(ot[:], ctxs_t[0:DH, :], rrb[:],
                                        mybir.AluOpType.mult)
                nc.sync.dma_start(
                    out.ap()[h * DH:(h + 1) * DH, fb * FB:(fb + 1) * FB],
                    ot[:])

            segs = [(0, 0), (1, 0), (0, 1), (1, 1),
                    (2, 0), (3, 0), (2, 1), (3, 1),
                    (4, 0), (5, 0), (4, 1), (5, 1)]
            for si, (h, fb) in enumerate(segs):
                ctx_ps = ps_c.tile([DH + 1, FB], dt.float32, tag="c",
                                   name="ctx_ps")
                pend = deque()
                first = (si == 0)
                for ti in range(NT):
                    s_ps = emit_scores(h, fb, ti)
                    et = emit_exp(s_ps, ti)
                    pend.append((ti, et))
                    # ctx at lag>=1 (never same-slot: PE would stall on the
                    # exp); seg 0 defers until vp-h0 exists (slot 8+).
                    if first and ti < 8:
                        nctx = 0
                    else:
                        nctx = 2 if len(pend) > 2 else (
                            1 if len(pend) == 2 else 0)
                    for _ in range(nctx):
                        tj, etj = pend.popleft()
                        emit_ctx(ctx_ps, h, tj, etj)
                    # fill: ~6 proj chunks per segment, front-loaded in
                    # seg 0 (vf0+k0c23 must be emitted before ctx/scores
                    # that depend on them — PE executes in order).
                    if si == 0:
                        if ti < 6:
                            pop_fill(1)
                    elif ti % 3 == 1:
                        pop_fill(1)
                while pend:
                    tj, etj = pend.popleft()
                    emit_ctx(ctx_ps, h, tj, etj)
                # drain + normalize (off the PE critical path)
                ctxs_t = fin.tile([DH + 1, FB], dt.float32, tag="ctxs",
                                  name="ctxs")
                nc.vector.tensor_copy(ctxs_t[:], ctx_ps[:])
                normalize(h, fb, ctxs_t)
            while fills:
                fills.popleft()()

    nc.compile()
    return nc


def _prep_core_inputs(c, x, Wq, bq, Wk, bk, Wv, bv, mask, ones_mask):
    bf16 = ml_dtypes.bfloat16
    b, hg = c // 2, c % 2
    cols = slice(hg * DL, (hg + 1) * DL)

    xT_ = np.ascontiguousarray(x[b].T.astype(bf16))

    def aug(W, bias):
        w = np.empty((D + 1, DL), dtype=bf16)
        w[:D] = W[:, cols].astype(bf16)
        w[D] = bias[cols].astype(bf16)
        return w

    if ones_mask:
        adder_t = np.zeros((P, NT), dtype=np.float32)
    else:
        add = ((mask[b].astype(np.float32) - 1.0) * 10000.0)
        if TOK_INTERLEAVED:
            adder_t = add.reshape(P, NT).copy()      # [p, ti] = add[p*16+ti]
        else:
            adder_t = add.reshape(NT, P).T.copy()    # [p, ti] = add[ti*128+p]

    return {"xT": xT_, "wq": aug(Wq, bq), "wk": aug(Wk, bk),
            "wv": aug(Wv, bv),
            "adder": np.ascontiguousarray(adder_t, dtype=np.float32)}


def kernel(x, Wq, bq, Wk, bk, Wv, bv, mask, _trace=False):
    from concourse.bass_utils import run_bass_kernel_spmd

    x = np.asarray(x, dtype=np.float32)
    Wq = np.asarray(Wq, dtype=np.float32)
    bq = np.asarray(bq, dtype=np.float32)
    Wk = np.asarray(Wk, dtype=np.float32)
    bk = np.asarray(bk, dtype=np.float32)
    Wv = np.asarray(Wv, dtype=np.float32)
    bv = np.asarray(bv, dtype=np.float32)
    mask = np.asarray(mask)

    with_bias = bool(bq.any() or bk.any() or bv.any())
    ones_mask = bool((mask == 1).all())
    key = ("nc", with_bias, ones_mask)
    if key not in _CACHE:
        _CACHE[key] = _build(with_bias=with_bias, ones_mask=ones_mask)
    nc = _CACHE[key]

    in_maps = [_prep_core_inputs(c, x, Wq, bq, Wk, bk, Wv, bv, mask,
                                 ones_mask)
               for c in range(NCORES)]
    res = run_bass_kernel_spmd(nc, in_maps, core_ids=list(range(NCORES)),
                               trace=_trace)
    if _trace:
        _CACHE["last_result"] = res

    full = np.empty((B, S, D), dtype=np.float32)
    for c in range(NCORES):
        b, hg = c // 2, c % 2
        full[b, :, hg * DL:(hg + 1) * DL] = res.results[c]["out"].T
    return full


# revision 7
# speedup vs baseline: 1.4850x; 1.4850x over previous
"""Self-contained Trainium2 Bass kernel for a 12-head attention layer.

Problem: x[4,2048,768] -> attention(QKV projections, softmax, context),
NUM_HEADS=12, SIZE_PER_HEAD=64, additive mask from mask[4,2048].

Sharding over 8 NeuronCores: core c handles batch b=c//2 and head-group
hg=c%2 (6 heads, 384 feature columns).  Everything is local per core:
no collectives.

Design (v2):
  - Projections Q^T,K^T [384,2048] and V^T feature-major, all bf16 matmuls
    at full K=128/M=128 utilization; V^T is DMA-XBAR-transposed to a
    token-major vp [128, 16T, 6x(64+1)] layout (65th col = ones, memset)
    so the context matmul's 65th row yields the softmax denominator.
  - Scores: fp8(e4m3) DoubleRow matmuls (2 contraction values per PE
    cell: lhsT/rhs [32,2,*]) -> 2x PE throughput vs bf16 at K=64.  The
    1/sqrt(64) scale is folded into the exp activation's scale, so Q/K
    are quantized at natural scale.  Rel-err budget ~1.3% < 2e-2 gate.
  - Softmax: exp on ACT (the hard bottleneck: 192 x [128,1024] tiles
    ~206us); mask enters as per-partition bias (all-ones mask -> bias 0).
  - Single-head segments (6 heads x 2 F-blocks x 16 T-tiles), scores
    psum double-buffered, ctx psum single [65,1024] + copy-drain;
    projections interleaved as fill so ACT never starves.
  - Normalize per segment: DMA-gather denominators -> DVE reciprocal ->
    DMA scatter -> GpSimd partition_broadcast -> DVE multiply -> DMA out.

Output per core: ctx^T [384,2048] f32; host transposes to [4,2048,768].
"""

import numpy as np
import ml_dtypes

B, S, D = 4, 2048, 768
H, DH = 12, 64
HL = 6          # heads per core
DL = HL * DH    # 384 feature columns per core
NCORES = 8
P = 128
KO = 6          # full k-subtiles of the 768 contraction
NT = S // P     # 16 T-tiles
FB = 1024       # F block size
NFB = S // FB   # 2 F blocks

SCORES_FP8 = False
# token layout inside a T-tile, set by the V transpose DMA semantics:
# True: vp[p, mt, :] holds token p*NT + mt  (XBAR row-major order)
# False: vp[p, mt, :] holds token mt*P + p  (natural; probe-verified)
TOK_INTERLEAVED = False

FP8 = ml_dtypes.float8_e4m3

_CACHE = {}


def _build(with_bias=False, ones_mask=True):
    import concourse.mybir as mybir
    import concourse.tile as tile
    from concourse import bacc

    dt = mybir.dt
    Exp = mybir.ActivationFunctionType.Exp
    DR = mybir.MatmulPerfMode.DoubleRow

    nc = bacc.Bacc("TRN2", target_bir_lowering=False, debug=False,
                   num_devices=NCORES)

    xT = nc.dram_tensor("xT", [D, S], dt.bfloat16, kind="ExternalInput")
    wq = nc.dram_tensor("wq", [D + 1, DL], dt.bfloat16, kind="ExternalInput")
    wk = nc.dram_tensor("wk", [D + 1, DL], dt.bfloat16, kind="ExternalInput")
    wv = nc.dram_tensor("wv", [D + 1, DL], dt.bfloat16, kind="ExternalInput")
    adder = nc.dram_tensor("adder", [P, NT], dt.float32, kind="ExternalInput")
    out = nc.dram_tensor("out", [DL, S], dt.float32, kind="ExternalOutput")

    KE = KO + 1 if with_bias else KO

    with tile.TileContext(nc) as tc:
        with (
            tc.tile_pool(name="persist", bufs=1) as sb,
            tc.tile_pool(name="etp", bufs=10) as etp,
            tc.tile_pool(name="stage", bufs=3) as stg,
            tc.tile_pool(name="vfp", bufs=2) as vfp,
            tc.tile_pool(name="fin", bufs=2) as fin,
            tc.tile_pool(name="ps_s", bufs=2, space="PSUM") as ps_s,
            tc.tile_pool(name="ps_c", bufs=1, space="PSUM") as ps_c,
            tc.tile_pool(name="ps_p", bufs=2, space="PSUM") as ps_p,
        ):
            # ---------------- input DMA (priority order) ----------------
            wqs = sb.tile([P, KO + 1, DL], dt.bfloat16, tag="wqs")
            wks = sb.tile([P, KO + 1, DL], dt.bfloat16, tag="wks")
            wvs = sb.tile([P, KO + 1, DL], dt.bfloat16, tag="wvs")
            xTs = sb.tile([P, KO + 1, S], dt.bfloat16, tag="xTs")
            adder_sb = sb.tile([P, NT], dt.float32, tag="adder")

            def dma_w(w_dram, w_sb, c0, c1):
                nc.sync.dma_start(
                    w_sb[:, 0:KO, c0:c1],
                    w_dram.ap()[0:D, c0:c1].rearrange(
                        "(ko p) m -> p ko m", p=P))
                if with_bias:
                    nc.sync.dma_start(w_sb[0:1, KO, c0:c1],
                                      w_dram.ap()[D:D + 1, c0:c1])

            def dma_x(s0, s1):
                nc.sync.dma_start(
                    xTs[:, 0:KO, s0:s1],
                    xT.ap()[:, s0:s1].rearrange("(ko p) s -> p ko s", p=P))

            dma_w(wq, wqs, 0, P)
            dma_w(wk, wks, 0, P)
            dma_x(0, FB)
            dma_w(wv, wvs, 0, DL)
            dma_x(FB, S)
            dma_w(wq, wqs, P, DL)
            dma_w(wk, wks, P, DL)
            if not ones_mask:
                nc.sync.dma_start(adder_sb[:], adder.ap())
            if with_bias:
                nc.gpsimd.memset(xTs[0:1, KO, :], 1.0)

            # persistent projection outputs
            if SCORES_FP8:
                qt8 = sb.tile([32, HL, 2, S], dt.float8e4, tag="qt8")
                kt8 = sb.tile([32, HL, 2, S], dt.float8e4, tag="kt8")
            else:
                qtb = sb.tile([P, 3, S], dt.bfloat16, tag="qtb")
                ktb = sb.tile([P, 3, S], dt.bfloat16, tag="ktb")
            vp = sb.tile([P, NT, HL * P], dt.bfloat16, tag="vp")
            for h in range(HL):
                nc.gpsimd.memset(vp[:, :, h * P + DH:h * P + DH + 1], 1.0)
                nc.gpsimd.memset(vp[:, :, h * P + DH + 1:(h + 1) * P], 0.0)

            # ---------------- projection helpers ----------------
            # one 512-wide proj chunk: psum <- W_mtile^T @ x_chunk
            def proj_mm(w_sb, mt, c):
                pt = ps_p.tile([P, 512], dt.float32, tag="p", name="pt")
                for k in range(KE):
                    lhsT = (w_sb[:, k, mt * P:(mt + 1) * P] if k < KO
                            else w_sb[0:1, k, mt * P:(mt + 1) * P])
                    rhs = (xTs[:, k, c * 512:(c + 1) * 512] if k < KO
                           else xTs[0:1, k, c * 512:(c + 1) * 512])
                    nc.tensor.matmul(pt[:], lhsT, rhs,
                                     start=(k == 0), stop=(k == KE - 1))
                return pt

            qk_stage = {}

            def qk_chunk(which, mt, c):
                # proj + fp8 cast; on odd c, fold-DMA the completed S-half
                # into the DoubleRow layout [32, h, j, S].
                w_sb = wqs if which == "q" else wks
                pt = proj_mm(w_sb, mt, c)
                if SCORES_FP8:
                    key = (which, mt, c // 2)
                    if c % 2 == 0:
                        qk_stage[key] = stg.tile([P, FB], dt.float8e4,
                                                 tag="stg", name="stg")
                    st = qk_stage[key]
                    nc.vector.tensor_copy(st[:, (c % 2) * 512:
                                             (c % 2) * 512 + 512], pt[:])
                    if c % 2 == 1:
                        dstt = qt8 if which == "q" else kt8
                        half = c // 2
                        for hip in range(2):
                            h = 2 * mt + hip
                            for j in range(2):
                                nc.sync.dma_start(
                                    dstt[0:32, h, j,
                                         half * FB:(half + 1) * FB],
                                    st[hip * 64 + 32 * j:
                                       hip * 64 + 32 * j + 32, :])
                        del qk_stage[key]
                else:
                    dstt = qtb if which == "q" else ktb
                    nc.vector.tensor_copy(
                        dstt[:, mt, c * 512:(c + 1) * 512], pt[:])

            vf_stage = {}

            def vf_chunk(mt, c):
                # feature-major V' proj chunk; after chunk 3, XBAR-transpose
                # both heads of this m-tile into token-major vp.
                pt = proj_mm(wvs, mt, c)
                if c == 0:
                    vf_stage[mt] = vfp.tile([P, S], dt.bfloat16,
                                            tag="vf", name="vf")
                vt = vf_stage[mt]
                nc.vector.tensor_copy(vt[:, c * 512:(c + 1) * 512], pt[:])
                if c == 3:
                    for hip in range(2):
                        h = 2 * mt + hip
                        # XBAR transpose needs a contiguous destination
                        # (strided dst slices produce wrong output on HW);
                        # bounce through a scratch tile, GpSimd copies into
                        # the strided vp layout.
                        vph = vfp.tile([P, NT, DH], dt.bfloat16,
                                       tag="vph", name="vph")
                        nc.sync.dma_start_transpose(
                            vph[:], vt[hip * DH:(hip + 1) * DH, :])
                        nc.gpsimd.tensor_copy(
                            vp[:, :, h * P:h * P + DH], vph[:])
                    del vf_stage[mt]

            # ---------------- warmup (HAM un-throttle during DMA) --------
            warm = sb.tile([P, 512], dt.bfloat16, tag="warm")
            nc.gpsimd.memset(warm[:], 0.0)
            wexp = sb.tile([P, 1], dt.bfloat16, tag="wexp")
            nc.scalar.activation(wexp[:], warm[:, 0:1], Exp)
            wpt = ps_s.tile([P, 512], dt.float32, tag="s", name="wpt")
            for wi in range(26):
                nc.tensor.matmul(wpt[:], warm[:, 0:P], warm[:],
                                 start=(wi == 0), stop=(wi == 25))

            # ---------------- prefix projections ----------------
            qk_chunk("q", 0, 0)
            qk_chunk("q", 0, 1)
            qk_chunk("k", 0, 0)
            qk_chunk("k", 0, 1)

            # ---------------- fill queue ----------------
            from collections import deque
            fills = deque()
            fills.append(lambda: vf_chunk(0, 0))
            fills.append(lambda: vf_chunk(0, 1))
            fills.append(lambda: qk_chunk("k", 0, 2))
            fills.append(lambda: qk_chunk("k", 0, 3))
            fills.append(lambda: vf_chunk(0, 2))
            fills.append(lambda: vf_chunk(0, 3))
            for mt in (1, 2):
                for c in range(4):
                    fills.append(lambda m=mt, cc=c: qk_chunk("q", m, cc))
                for c in range(4):
                    fills.append(lambda m=mt, cc=c: qk_chunk("k", m, cc))
                for c in range(4):
                    fills.append(lambda m=mt, cc=c: vf_chunk(m, cc))
            # q0c23 (needed by seg 2 = (h0,fb1)) goes right after seg0's
            # must-haves; m1/m2 chunks follow FIFO.
            fills.insert(6, lambda: qk_chunk("q", 0, 2))
            fills.insert(7, lambda: qk_chunk("q", 0, 3))

            def pop_fill(n):
                for _ in range(n):
                    if fills:
                        fills.popleft()()

            # ---------------- attention segments ----------------
            def emit_scores(h, fb, ti):
                s_ps = ps_s.tile([P, FB], dt.float32, tag="s", name="s_ps")
                if SCORES_FP8:
                    if TOK_INTERLEAVED:
                        ksel = kt8[0:32, h, :, :].rearrange(
                            "p j (t g) -> p j g t", g=NT)[:, :, ti, :]
                    else:
                        ksel = kt8[0:32, h, :, ti * P:(ti + 1) * P]
                    for n in range(2):
                        fc = fb * FB + n * 512
                        nc.tensor.matmul(
                            s_ps[:, n * 512:(n + 1) * 512], ksel,
                            qt8[0:32, h, :, fc:fc + 512],
                            start=True, stop=True, perf_mode=DR)
                else:
                    off = (h % 2) * DH
                    if TOK_INTERLEAVED:
                        ksel = ktb[off:off + DH, h // 2, :].rearrange(
                            "d (t g) -> d g t", g=NT)[:, ti, :]
                    else:
                        ksel = ktb[off:off + DH, h // 2,
                                   ti * P:(ti + 1) * P]
                    for n in range(2):
                        fc = fb * FB + n * 512
                        nc.tensor.matmul(
                            s_ps[:, n * 512:(n + 1) * 512], ksel,
                            qtb[off:off + DH, h // 2, fc:fc + 512],
                            start=True, stop=True)
                return s_ps

            def emit_exp(s_ps, ti):
                et = etp.tile([P, FB], dt.bfloat16, tag="et", name="et")
                bias = 0.0 if ones_mask else adder_sb[:, ti:ti + 1]
                nc.scalar.activation(et[:], s_ps[:], Exp,
                                     bias=bias, scale=0.125)
                return et

            def emit_ctx(ctx_ps, h, ti, et):
                for n in range(2):
                    nc.tensor.matmul(
                        ctx_ps[:, n * 512:(n + 1) * 512],
                        vp[:, ti, h * P:(h + 1) * P],
                        et[:, n * 512:(n + 1) * 512],
                        start=(ti == 0), stop=(ti == NT - 1))

            def normalize(h, fb, ctxs_t):
                nfb = FB // P   # 8 denominator cols
                den = fin.tile([P, nfb], dt.float32, tag="den", name="den")
                nc.sync.dma_start(den[:], ctxs_t[DH:DH + 1, :])
                rec = fin.tile([P, nfb], dt.float32, tag="rec", name="rec")
                nc.vector.reciprocal(rec[:], den[:])
                rr = fin.tile([1, FB], dt.float32, tag="rr", name="rr")
                nc.sync.dma_start(rr[:], rec[:])
                rrb = fin.tile([DH, FB], dt.float32, tag="rrb", name="rrb")
                nc.gpsimd.partition_broadcast(rrb[:], rr[:])
                ot = fin.tile([DH, FB], dt.float32, tag="ot", name="ot")
                nc.vector.tensor_tensor(ot[:], ctxs_t[0:DH, :], rrb[:],
                                        mybir.AluOpType.mult)
                nc.sync.dma_start(
                    out.ap()[h * DH:(h + 1) * DH, fb * FB:(fb + 1) * FB],
                    ot[:])

            segs = [(0, 0), (1, 0), (0, 1), (1, 1),
                    (2, 0), (3, 0), (2, 1), (3, 1),
                    (4, 0), (5, 0), (4, 1), (5, 1)]
            for si, (h, fb) in enumerate(segs):
                ctx_ps = ps_c.tile([P, FB], dt.float32, tag="c",
                                   name="ctx_ps")
                pend = deque()
                first = (si == 0)
                for ti in range(NT):
                    s_ps = emit_scores(h, fb, ti)
                    et = emit_exp(s_ps, ti)
                    pend.append((ti, et))
                    # ctx at lag>=1 (never same-slot: PE would stall on the
                    # exp); seg 0 defers until vp-h0 exists (slot 8+).
                    if first and ti < 8:
                        nctx = 0
                    else:
                        nctx = 2 if len(pend) > 2 else (
                            1 if len(pend) == 2 else 0)
                    for _ in range(nctx):
                        tj, etj = pend.popleft()
                        emit_ctx(ctx_ps, h, tj, etj)
                    # fill: ~6 proj chunks per segment, front-loaded in
                    # seg 0 (vf0+k0c23 must be emitted before ctx/scores
                    # that depend on them — PE executes in order).
                    if si == 0:
                        if ti < 6:
                            pop_fill(1)
                    elif ti % 3 == 1:
                        pop_fill(1)
                while pend:
                    tj, etj = pend.popleft()
                    emit_ctx(ctx_ps, h, tj, etj)
                # drain + normalize (off the PE critical path)
                ctxs_t = fin.tile([DH + 1, FB], dt.float32, tag="ctxs",
                                  name="ctxs")
                nc.vector.tensor_copy(ctxs_t[:], ctx_ps[0:DH + 1, :])
                normalize(h, fb, ctxs_t)
            while fills:
                fills.popleft()()

    nc.compile()
    return nc


def _prep_core_inputs(c, x, Wq, bq, Wk, bk, Wv, bv, mask, ones_mask):
    bf16 = ml_dtypes.bfloat16
    b, hg = c // 2, c % 2
    cols = slice(hg * DL, (hg + 1) * DL)

    xT_ = np.ascontiguousarray(x[b].T.astype(bf16))

    def aug(W, bias):
        w = np.empty((D + 1, DL), dtype=bf16)
        w[:D] = W[:, cols].astype(bf16)
        w[D] = bias[cols].astype(bf16)
        return w

    if ones_mask:
        adder_t = np.zeros((P, NT), dtype=np.float32)
    else:
        add = ((mask[b].astype(np.float32) - 1.0) * 10000.0)
        if TOK_INTERLEAVED:
            adder_t = add.reshape(P, NT).copy()      # [p, ti] = add[p*16+ti]
        else:
            adder_t = add.reshape(NT, P).T.copy()    # [p, ti] = add[ti*128+p]

    return {"xT": xT_, "wq": aug(Wq, bq), "wk": aug(Wk, bk),
            "wv": aug(Wv, bv),
            "adder": np.ascontiguousarray(adder_t, dtype=np.float32)}


def kernel(x, Wq, bq, Wk, bk, Wv, bv, mask, _trace=False):
    from concourse.bass_utils import run_bass_kernel_spmd

    x = np.asarray(x, dtype=np.float32)
    Wq = np.asarray(Wq, dtype=np.float32)
    bq = np.asarray(bq, dtype=np.float32)
    Wk = np.asarray(Wk, dtype=np.float32)
    bk = np.asarray(bk, dtype=np.float32)
    Wv = np.asarray(Wv, dtype=np.float32)
    bv = np.asarray(bv, dtype=np.float32)
    mask = np.asarray(mask)

    with_bias = bool(bq.any() or bk.any() or bv.any())
    ones_mask = bool((mask == 1).all())
    key = ("nc", with_bias, ones_mask)
    if key not in _CACHE:
        _CACHE[key] = _build(with_bias=with_bias, ones_mask=ones_mask)
    nc = _CACHE[key]

    in_maps = [_prep_core_inputs(c, x, Wq, bq, Wk, bk, Wv, bv, mask,
                                 ones_mask)
               for c in range(NCORES)]
    res = run_bass_kernel_spmd(nc, in_maps, core_ids=list(range(NCORES)),
                               trace=_trace)
    if _trace:
        _CACHE["last_result"] = res

    full = np.empty((B, S, D), dtype=np.float32)
    for c in range(NCORES):
        b, hg = c // 2, c % 2
        full[b, :, hg * DL:(hg + 1) * DL] = res.results[c]["out"].T
    return full


# revision 10
# speedup vs baseline: 1.5127x; 1.0187x over previous
"""Self-contained Trainium2 Bass kernel for a 12-head attention layer.

Problem: x[4,2048,768] -> attention(QKV projections, softmax, context),
NUM_HEADS=12, SIZE_PER_HEAD=64, additive mask from mask[4,2048].

Sharding over 8 NeuronCores: core c handles batch b=c//2 and head-group
hg=c%2 (6 heads, 384 feature columns).  Everything is local per core:
no collectives.

Design (v2):
  - Projections Q^T,K^T [384,2048] and V^T feature-major, all bf16 matmuls
    at full K=128/M=128 utilization; V^T is DMA-XBAR-transposed to a
    token-major vp [128, 16T, 6x(64+1)] layout (65th col = ones, memset)
    so the context matmul's 65th row yields the softmax denominator.
  - Scores: fp8(e4m3) DoubleRow matmuls (2 contraction values per PE
    cell: lhsT/rhs [32,2,*]) -> 2x PE throughput vs bf16 at K=64.  The
    1/sqrt(64) scale is folded into the exp activation's scale, so Q/K
    are quantized at natural scale.  Rel-err budget ~1.3% < 2e-2 gate.
  - Softmax: exp on ACT (the hard bottleneck: 192 x [128,1024] tiles
    ~206us); mask enters as per-partition bias (all-ones mask -> bias 0).
  - Single-head segments (6 heads x 2 F-blocks x 16 T-tiles), scores
    psum double-buffered, ctx psum single [65,1024] + copy-drain;
    projections interleaved as fill so ACT never starves.
  - Normalize per segment: DMA-gather denominators -> DVE reciprocal ->
    DMA scatter -> GpSimd partition_broadcast -> DVE multiply -> DMA out.

Output per core: ctx^T [384,2048] f32; host transposes to [4,2048,768].
"""

import numpy as np
import ml_dtypes

B, S, D = 4, 2048, 768
H, DH = 12, 64
HL = 6          # heads per core
DL = HL * DH    # 384 feature columns per core
NCORES = 8
P = 128
KO = 6          # full k-subtiles of the 768 contraction
NT = S // P     # 16 T-tiles
FB = 1024       # F block size
NFB = S // FB   # 2 F blocks

SCORES_FP8 = False
# token layout inside a T-tile, set by the V transpose DMA semantics:
# True: vp[p, mt, :] holds token p*NT + mt  (XBAR row-major order)
# False: vp[p, mt, :] holds token mt*P + p  (natural; probe-verified)
TOK_INTERLEAVED = False

FP8 = ml_dtypes.float8_e4m3

_CACHE = {}


def _build(with_bias=False, ones_mask=True):
    import concourse.mybir as mybir
    import concourse.tile as tile
    from concourse import bacc

    dt = mybir.dt
    Exp = mybir.ActivationFunctionType.Exp
    DR = mybir.MatmulPerfMode.DoubleRow

    nc = bacc.Bacc("TRN2", target_bir_lowering=False, debug=False,
                   num_devices=NCORES)

    xT = nc.dram_tensor("xT", [D, S], dt.bfloat16, kind="ExternalInput")
    wq = nc.dram_tensor("wq", [D + 1, DL], dt.bfloat16, kind="ExternalInput")
    wk = nc.dram_tensor("wk", [D + 1, DL], dt.bfloat16, kind="ExternalInput")
    wv = nc.dram_tensor("wv", [D + 1, DL], dt.bfloat16, kind="ExternalInput")
    adder = nc.dram_tensor("adder", [P, NT], dt.float32, kind="ExternalInput")
    out = nc.dram_tensor("out", [DL, S], dt.float32, kind="ExternalOutput")

    KE = KO + 1 if with_bias else KO

    with tile.TileContext(nc) as tc:
        with (
            tc.tile_pool(name="persist", bufs=1) as sb,
            tc.tile_pool(name="etp", bufs=10) as etp,
            tc.tile_pool(name="stage", bufs=3) as stg,
            tc.tile_pool(name="vfp", bufs=2) as vfp,
            tc.tile_pool(name="fin", bufs=2) as fin,
            tc.tile_pool(name="ps_s", bufs=2, space="PSUM") as ps_s,
            tc.tile_pool(name="ps_c", bufs=1, space="PSUM") as ps_c,
            tc.tile_pool(name="ps_p", bufs=2, space="PSUM") as ps_p,
        ):
            # ---------------- input DMA (priority order) ----------------
            wqs = sb.tile([P, KO + 1, DL], dt.bfloat16, tag="wqs")
            wks = sb.tile([P, KO + 1, DL], dt.bfloat16, tag="wks")
            wvs = sb.tile([P, KO + 1, DL], dt.bfloat16, tag="wvs")
            xTs = sb.tile([P, KO + 1, S], dt.bfloat16, tag="xTs")
            adder_sb = sb.tile([P, NT], dt.float32, tag="adder")

            def dma_w(w_dram, w_sb, c0, c1):
                nc.sync.dma_start(
                    w_sb[:, 0:KO, c0:c1],
                    w_dram.ap()[0:D, c0:c1].rearrange(
                        "(ko p) m -> p ko m", p=P))
                if with_bias:
                    nc.sync.dma_start(w_sb[0:1, KO, c0:c1],
                                      w_dram.ap()[D:D + 1, c0:c1])

            def dma_x(s0, s1):
                nc.sync.dma_start(
                    xTs[:, 0:KO, s0:s1],
                    xT.ap()[:, s0:s1].rearrange("(ko p) s -> p ko s", p=P))

            dma_w(wq, wqs, 0, P)
            dma_w(wk, wks, 0, P)
            dma_x(0, FB)
            dma_w(wv, wvs, 0, DL)
            dma_x(FB, S)
            dma_w(wq, wqs, P, DL)
            dma_w(wk, wks, P, DL)
            if not ones_mask:
                nc.sync.dma_start(adder_sb[:], adder.ap())
            if with_bias:
                nc.gpsimd.memset(xTs[0:1, KO, :], 1.0)

            # persistent projection outputs
            if SCORES_FP8:
                qt8 = sb.tile([32, HL, 2, S], dt.float8e4, tag="qt8")
                kt8 = sb.tile([32, HL, 2, S], dt.float8e4, tag="kt8")
            else:
                qtb = sb.tile([P, 3, S], dt.bfloat16, tag="qtb")
                ktb = sb.tile([P, 3, S], dt.bfloat16, tag="ktb")
            vp = sb.tile([P, NT, HL * P], dt.bfloat16, tag="vp")
            for h in range(HL):
                nc.gpsimd.memset(vp[:, :, h * P + DH:h * P + DH + 1], 1.0)
                nc.gpsimd.memset(vp[:, :, h * P + DH + 1:(h + 1) * P], 0.0)

            # ---------------- projection helpers ----------------
            # one 512-wide proj chunk: psum <- W_mtile^T @ x_chunk
            def proj_mm(w_sb, mt, c):
                pt = ps_p.tile([P, 512], dt.float32, tag="p", name="pt")
                for k in range(KE):
                    lhsT = (w_sb[:, k, mt * P:(mt + 1) * P] if k < KO
                            else w_sb[0:1, k, mt * P:(mt + 1) * P])
                    rhs = (xTs[:, k, c * 512:(c + 1) * 512] if k < KO
                           else xTs[0:1, k, c * 512:(c + 1) * 512])
                    nc.tensor.matmul(pt[:], lhsT, rhs,
                                     start=(k == 0), stop=(k == KE - 1))
                return pt

            qk_stage = {}

            def qk_chunk(which, mt, c):
                # proj + fp8 cast; on odd c, fold-DMA the completed S-half
                # into the DoubleRow layout [32, h, j, S].
                w_sb = wqs if which == "q" else wks
                pt = proj_mm(w_sb, mt, c)
                if SCORES_FP8:
                    key = (which, mt, c // 2)
                    if c % 2 == 0:
                        qk_stage[key] = stg.tile([P, FB], dt.float8e4,
                                                 tag="stg", name="stg")
                    st = qk_stage[key]
                    nc.vector.tensor_copy(st[:, (c % 2) * 512:
                                             (c % 2) * 512 + 512], pt[:])
                    if c % 2 == 1:
                        dstt = qt8 if which == "q" else kt8
                        half = c // 2
                        for hip in range(2):
                            h = 2 * mt + hip
                            for j in range(2):
                                nc.sync.dma_start(
                                    dstt[0:32, h, j,
                                         half * FB:(half + 1) * FB],
                                    st[hip * 64 + 32 * j:
                                       hip * 64 + 32 * j + 32, :])
                        del qk_stage[key]
                else:
                    dstt = qtb if which == "q" else ktb
                    nc.vector.tensor_copy(
                        dstt[:, mt, c * 512:(c + 1) * 512], pt[:])

            vf_stage = {}

            def vf_chunk(mt, c):
                # feature-major V' proj chunk; after chunk 3, XBAR-transpose
                # both heads of this m-tile into token-major vp.
                pt = proj_mm(wvs, mt, c)
                if c == 0:
                    vf_stage[mt] = vfp.tile([P, S], dt.bfloat16,
                                            tag="vf", name="vf")
                vt = vf_stage[mt]
                nc.vector.tensor_copy(vt[:, c * 512:(c + 1) * 512], pt[:])
                if c == 3:
                    for hip in range(2):
                        h = 2 * mt + hip
                        # XBAR transpose needs a contiguous destination
                        # (strided dst slices produce wrong output on HW);
                        # bounce through a scratch tile, GpSimd copies into
                        # the strided vp layout.
                        vph = vfp.tile([P, NT, DH], dt.bfloat16,
                                       tag="vph", name="vph")
                        nc.sync.dma_start_transpose(
                            vph[:], vt[hip * DH:(hip + 1) * DH, :])
                        nc.gpsimd.tensor_copy(
                            vp[:, :, h * P:h * P + DH], vph[:])
                    del vf_stage[mt]

            # ---------------- warmup (HAM un-throttle during DMA) --------
            # runs on uninitialized SBUF (results never read; psum slot is
            # reclaimed via start=True) so the PE starts at t~0 with no
            # upstream deps; psum from the proj pool so the scores pool
            # rotation keeps its full 2-slot lag.
            warm = sb.tile([P, 512], dt.bfloat16, tag="warm")
            nc.vector.memset(warm[:], 0.0)
            wexp = sb.tile([P, 1], dt.bfloat16, tag="wexp")
            nc.scalar.activation(wexp[:], warm[:, 0:1], Exp)
            ones1 = sb.tile([1, DH], dt.bfloat16, tag="ones1")
            nc.gpsimd.memset(ones1[:], 1.0)
            wpt = ps_p.tile([P, 512], dt.float32, tag="p", name="wpt")
            for wi in range(26):
                nc.tensor.matmul(wpt[:], warm[:, 0:P], warm[:],
                                 start=(wi == 0), stop=(wi == 25))

            # ---------------- prefix projections ----------------
            qk_chunk("q", 0, 0)
            qk_chunk("q", 0, 1)
            qk_chunk("k", 0, 0)
            qk_chunk("k", 0, 1)

            # ---------------- fill queue ----------------
            from collections import deque
            fills = deque()
            fills.append(lambda: vf_chunk(0, 0))
            fills.append(lambda: vf_chunk(0, 1))
            fills.append(lambda: qk_chunk("k", 0, 2))
            fills.append(lambda: qk_chunk("k", 0, 3))
            fills.append(lambda: vf_chunk(0, 2))
            fills.append(lambda: vf_chunk(0, 3))
            for mt in (1, 2):
                for c in range(4):
                    fills.append(lambda m=mt, cc=c: qk_chunk("q", m, cc))
                for c in range(4):
                    fills.append(lambda m=mt, cc=c: qk_chunk("k", m, cc))
                for c in range(4):
                    fills.append(lambda m=mt, cc=c: vf_chunk(m, cc))
            # q0c23 (needed by seg 2 = (h0,fb1)) goes right after seg0's
            # must-haves; m1/m2 chunks follow FIFO.
            fills.insert(6, lambda: qk_chunk("q", 0, 2))
            fills.insert(7, lambda: qk_chunk("q", 0, 3))

            def pop_fill(n):
                for _ in range(n):
                    if fills:
                        fills.popleft()()

            # ---------------- attention segments ----------------
            def emit_scores(h, fb, ti):
                s_ps = ps_s.tile([P, FB], dt.float32, tag="s", name="s_ps")
                if SCORES_FP8:
                    if TOK_INTERLEAVED:
                        ksel = kt8[0:32, h, :, :].rearrange(
                            "p j (t g) -> p j g t", g=NT)[:, :, ti, :]
                    else:
                        ksel = kt8[0:32, h, :, ti * P:(ti + 1) * P]
                    for n in range(2):
                        fc = fb * FB + n * 512
                        nc.tensor.matmul(
                            s_ps[:, n * 512:(n + 1) * 512], ksel,
                            qt8[0:32, h, :, fc:fc + 512],
                            start=True, stop=True, perf_mode=DR)
                else:
                    off = (h % 2) * DH
                    if TOK_INTERLEAVED:
                        ksel = ktb[off:off + DH, h // 2, :].rearrange(
                            "d (t g) -> d g t", g=NT)[:, ti, :]
                    else:
                        ksel = ktb[off:off + DH, h // 2,
                                   ti * P:(ti + 1) * P]
                    for n in range(2):
                        fc = fb * FB + n * 512
                        nc.tensor.matmul(
                            s_ps[:, n * 512:(n + 1) * 512], ksel,
                            qtb[off:off + DH, h // 2, fc:fc + 512],
                            start=True, stop=True)
                return s_ps

            def emit_exp(s_ps, ti):
                et = etp.tile([P, FB], dt.bfloat16, tag="et", name="et")
                bias = 0.0 if ones_mask else adder_sb[:, ti:ti + 1]
                nc.scalar.activation(et[:], s_ps[:], Exp,
                                     bias=bias, scale=0.125)
                return et

            def emit_ctx(ctx_ps, h, ti, et):
                for n in range(2):
                    nc.tensor.matmul(
                        ctx_ps[:, n * 512:(n + 1) * 512],
                        vp[:, ti, h * P:(h + 1) * P],
                        et[:, n * 512:(n + 1) * 512],
                        start=(ti == 0), stop=(ti == NT - 1))

            def normalize(h, fb, ctxs_t):
                nfb = FB // P   # 8 denominator cols
                den = fin.tile([P, nfb], dt.float32, tag="den", name="den")
                nc.sync.dma_start(den[:], ctxs_t[DH:DH + 1, :])
                rec = fin.tile([P, nfb], dt.float32, tag="rec", name="rec")
                nc.vector.reciprocal(rec[:], den[:])
                rr = fin.tile([1, FB], dt.float32, tag="rr", name="rr")
                nc.sync.dma_start(rr[:], rec[:])
                rrb = fin.tile([DH, FB], dt.float32, tag="rrb", name="rrb")
                nc.gpsimd.partition_broadcast(rrb[:], rr[:])
                ot = fin.tile([DH, FB], dt.float32, tag="ot", name="ot")
                nc.vector.tensor_tensor(ot[:], ctxs_t[0:DH, :], rrb[:],
                                        mybir.AluOpType.mult)
                nc.sync.dma_start(
                    out.ap()[h * DH:(h + 1) * DH, fb * FB:(fb + 1) * FB],
                    ot[:])

            segs = [(0, 0), (1, 0), (0, 1), (1, 1),
                    (2, 0), (3, 0), (2, 1), (3, 1),
                    (4, 0), (5, 0), (4, 1), (5, 1)]
            for si, (h, fb) in enumerate(segs):
                ctx_ps = ps_c.tile([P, FB], dt.float32, tag="c",
                                   name="ctx_ps")
                pend = deque()
                first = (si == 0)
                for ti in range(NT):
                    s_ps = emit_scores(h, fb, ti)
                    et = emit_exp(s_ps, ti)
                    pend.append((ti, et))
                    # ctx at lag>=1 (never same-slot: PE would stall on the
                    # exp); seg 0 defers until vp-h0 exists (slot 8+).
                    if first and ti < 8:
                        nctx = 0
                    else:
                        nctx = 2 if len(pend) > 2 else (
                            1 if len(pend) == 2 else 0)
                    for _ in range(nctx):
                        tj, etj = pend.popleft()
                        emit_ctx(ctx_ps, h, tj, etj)
                    # fill: ~6 proj chunks per segment, front-loaded in
                    # seg 0 (vf0+k0c23 must be emitted before ctx/scores
                    # that depend on them — PE executes in order).
                    if si == 0:
                        if ti < 6:
                            pop_fill(1)
                    elif ti % 3 == 1:
                        pop_fill(1)
                while pend:
                    tj, etj = pend.popleft()
                    emit_ctx(ctx_ps, h, tj, etj)
                # drain + normalize (off the PE critical path)
                ctxs_t = fin.tile([DH + 1, FB], dt.float32, tag="ctxs",
                                  name="ctxs")
                nc.vector.tensor_copy(ctxs_t[:], ctx_ps[0:DH + 1, :])
                if si < len(segs) - 1:
                    normalize(h, fb, ctxs_t)
                else:
                    # tail: minimize serial latency (no DMA gather/scatter,
                    # no gpsimd broadcast): DVE row-reciprocal + PE ones-
                    # broadcast into psum + DVE multiply, 512-chunked.
                    rrow = fin.tile([1, FB], dt.bfloat16, tag="rrow",
                                    name="rrow")
                    with nc.allow_low_precision(
                            reason="bf16 reciprocal feeds the PE "
                                   "broadcast; ~0.2% rel err is in budget"):
                        nc.vector.reciprocal(rrow[:], ctxs_t[DH:DH + 1, :])
                    for n in range(2):
                        rb = ps_s.tile([DH, 512], dt.float32, tag="s",
                                       name="rb")
                        nc.tensor.matmul(rb[:], ones1[:],
                                         rrow[:, n * 512:(n + 1) * 512],
                                         start=True, stop=True)
                        ot = fin.tile([DH, 512], dt.float32, tag="ot2",
                                      name="ot2")
                        nc.vector.tensor_tensor(
                            ot[:], ctxs_t[0:DH, n * 512:(n + 1) * 512],
                            rb[:], mybir.AluOpType.mult)
                        nc.sync.dma_start(
                            out.ap()[h * DH:(h + 1) * DH,
                                     fb * FB + n * 512:
                                     fb * FB + (n + 1) * 512],
                            ot[:])
            while fills:
                fills.popleft()()

    nc.compile()
    return nc


def _prep_core_inputs(c, x, Wq, bq, Wk, bk, Wv, bv, mask, ones_mask):
    bf16 = ml_dtypes.bfloat16
    b, hg = c // 2, c % 2
    cols = slice(hg * DL, (hg + 1) * DL)

    xT_ = np.ascontiguousarray(x[b].T.astype(bf16))

    def aug(W, bias):
        w = np.empty((D + 1, DL), dtype=bf16)
        w[:D] = W[:, cols].astype(bf16)
        w[D] = bias[cols].astype(bf16)
        return w

    if ones_mask:
        adder_t = np.zeros((P, NT), dtype=np.float32)
    else:
        add = ((mask[b].astype(np.float32) - 1.0) * 10000.0)
        if TOK_INTERLEAVED:
            adder_t = add.reshape(P, NT).copy()      # [p, ti] = add[p*16+ti]
        else:
            adder_t = add.reshape(NT, P).T.copy()    # [p, ti] = add[ti*128+p]

    return {"xT": xT_, "wq": aug(Wq, bq), "wk": aug(Wk, bk),
            "wv": aug(Wv, bv),
            "adder": np.ascontiguousarray(adder_t, dtype=np.float32)}


def kernel(x, Wq, bq, Wk, bk, Wv, bv, mask, _trace=False):
    from concourse.bass_utils import run_bass_kernel_spmd

    x = np.asarray(x, dtype=np.float32)
    Wq = np.asarray(Wq, dtype=np.float32)
    bq = np.asarray(bq, dtype=np.float32)
    Wk = np.asarray(Wk, dtype=np.float32)
    bk = np.asarray(bk, dtype=np.float32)
    Wv = np.asarray(Wv, dtype=np.float32)
    bv = np.asarray(bv, dtype=np.float32)
    mask = np.asarray(mask)

    with_bias = bool(bq.any() or bk.any() or bv.any())
    ones_mask = bool((mask == 1).all())
    key = ("nc", with_bias, ones_mask)
    if key not in _CACHE:
        _CACHE[key] = _build(with_bias=with_bias, ones_mask=ones_mask)
    nc = _CACHE[key]

    in_maps = [_prep_core_inputs(c, x, Wq, bq, Wk, bk, Wv, bv, mask,
                                 ones_mask)
               for c in range(NCORES)]
    res = run_bass_kernel_spmd(nc, in_maps, core_ids=list(range(NCORES)),
                               trace=_trace)
    if _trace:
        _CACHE["last_result"] = res

    full = np.empty((B, S, D), dtype=np.float32)
    for c in range(NCORES):
        b, hg = c // 2, c % 2
        full[b, :, hg * DL:(hg + 1) * DL] = res.results[c]["out"].T
    return full


# revision 11
# speedup vs baseline: 1.5306x; 1.0118x over previous
"""Self-contained Trainium2 Bass kernel for a 12-head attention layer.

Problem: x[4,2048,768] -> attention(QKV projections, softmax, context),
NUM_HEADS=12, SIZE_PER_HEAD=64, additive mask from mask[4,2048].

Sharding over 8 NeuronCores: core c handles batch b=c//2 and head-group
hg=c%2 (6 heads, 384 feature columns).  Everything is local per core:
no collectives.

Design (v2):
  - Projections Q^T,K^T [384,2048] and V^T feature-major, all bf16 matmuls
    at full K=128/M=128 utilization; V^T is DMA-XBAR-transposed to a
    token-major vp [128, 16T, 6x(64+1)] layout (65th col = ones, memset)
    so the context matmul's 65th row yields the softmax denominator.
  - Scores: fp8(e4m3) DoubleRow matmuls (2 contraction values per PE
    cell: lhsT/rhs [32,2,*]) -> 2x PE throughput vs bf16 at K=64.  The
    1/sqrt(64) scale is folded into the exp activation's scale, so Q/K
    are quantized at natural scale.  Rel-err budget ~1.3% < 2e-2 gate.
  - Softmax: exp on ACT (the hard bottleneck: 192 x [128,1024] tiles
    ~206us); mask enters as per-partition bias (all-ones mask -> bias 0).
  - Single-head segments (6 heads x 2 F-blocks x 16 T-tiles), scores
    psum double-buffered, ctx psum single [65,1024] + copy-drain;
    projections interleaved as fill so ACT never starves.
  - Normalize per segment: DMA-gather denominators -> DVE reciprocal ->
    DMA scatter -> GpSimd partition_broadcast -> DVE multiply -> DMA out.

Output per core: ctx^T [384,2048] f32; host transposes to [4,2048,768].
"""

import numpy as np
import ml_dtypes

B, S, D = 4, 2048, 768
H, DH = 12, 64
HL = 6          # heads per core
DL = HL * DH    # 384 feature columns per core
NCORES = 8
P = 128
KO = 6          # full k-subtiles of the 768 contraction
NT = S // P     # 16 T-tiles
FB = 1024       # F block size
NFB = S // FB   # 2 F blocks

SCORES_FP8 = False
# token layout inside a T-tile, set by the V transpose DMA semantics:
# True: vp[p, mt, :] holds token p*NT + mt  (XBAR row-major order)
# False: vp[p, mt, :] holds token mt*P + p  (natural; probe-verified)
TOK_INTERLEAVED = False

FP8 = ml_dtypes.float8_e4m3

_CACHE = {}


def _build(with_bias=False, ones_mask=True):
    import concourse.mybir as mybir
    import concourse.tile as tile
    from concourse import bacc

    dt = mybir.dt
    Exp = mybir.ActivationFunctionType.Exp
    DR = mybir.MatmulPerfMode.DoubleRow

    nc = bacc.Bacc("TRN2", target_bir_lowering=False, debug=False,
                   num_devices=NCORES)

    xT = nc.dram_tensor("xT", [D, S], dt.bfloat16, kind="ExternalInput")
    wq = nc.dram_tensor("wq", [D + 1, DL], dt.bfloat16, kind="ExternalInput")
    wk = nc.dram_tensor("wk", [D + 1, DL], dt.bfloat16, kind="ExternalInput")
    wv = nc.dram_tensor("wv", [D + 1, DL], dt.bfloat16, kind="ExternalInput")
    adder = nc.dram_tensor("adder", [P, NT], dt.float32, kind="ExternalInput")
    out = nc.dram_tensor("out", [DL, S], dt.float32, kind="ExternalOutput")

    KE = KO + 1 if with_bias else KO

    with tile.TileContext(nc) as tc:
        with (
            tc.tile_pool(name="persist", bufs=1) as sb,
            tc.tile_pool(name="etp", bufs=10) as etp,
            tc.tile_pool(name="stage", bufs=3) as stg,
            tc.tile_pool(name="vfp", bufs=2) as vfp,
            tc.tile_pool(name="fin", bufs=2) as fin,
            tc.tile_pool(name="ps_s", bufs=2, space="PSUM") as ps_s,
            tc.tile_pool(name="ps_c", bufs=1, space="PSUM") as ps_c,
            tc.tile_pool(name="ps_p", bufs=2, space="PSUM") as ps_p,
        ):
            # ---------------- input DMA (priority order) ----------------
            wqs = sb.tile([P, KO + 1, DL], dt.bfloat16, tag="wqs")
            wks = sb.tile([P, KO + 1, DL], dt.bfloat16, tag="wks")
            wvs = sb.tile([P, KO + 1, DL], dt.bfloat16, tag="wvs")
            xTs = sb.tile([P, KO + 1, S], dt.bfloat16, tag="xTs")
            adder_sb = sb.tile([P, NT], dt.float32, tag="adder")

            def dma_w(w_dram, w_sb, c0, c1):
                nc.sync.dma_start(
                    w_sb[:, 0:KO, c0:c1],
                    w_dram.ap()[0:D, c0:c1].rearrange(
                        "(ko p) m -> p ko m", p=P))
                if with_bias:
                    nc.sync.dma_start(w_sb[0:1, KO, c0:c1],
                                      w_dram.ap()[D:D + 1, c0:c1])

            def dma_x(s0, s1):
                nc.sync.dma_start(
                    xTs[:, 0:KO, s0:s1],
                    xT.ap()[:, s0:s1].rearrange("(ko p) s -> p ko s", p=P))

            dma_w(wq, wqs, 0, P)
            dma_w(wk, wks, 0, P)
            dma_x(0, FB)
            dma_w(wv, wvs, 0, DL)
            dma_x(FB, S)
            dma_w(wq, wqs, P, DL)
            dma_w(wk, wks, P, DL)
            if not ones_mask:
                nc.sync.dma_start(adder_sb[:], adder.ap())
            if with_bias:
                nc.gpsimd.memset(xTs[0:1, KO, :], 1.0)

            # persistent projection outputs
            if SCORES_FP8:
                qt8 = sb.tile([32, HL, 2, S], dt.float8e4, tag="qt8")
                kt8 = sb.tile([32, HL, 2, S], dt.float8e4, tag="kt8")
            else:
                qtb = sb.tile([P, 3, S], dt.bfloat16, tag="qtb")
                ktb = sb.tile([P, 3, S], dt.bfloat16, tag="ktb")
            vp = sb.tile([P, NT, HL * P], dt.bfloat16, tag="vp")
            for h in range(HL):
                nc.gpsimd.memset(vp[:, :, h * P + DH:h * P + DH + 1], 1.0)
                nc.gpsimd.memset(vp[:, :, h * P + DH + 1:(h + 1) * P], 0.0)

            # ---------------- projection helpers ----------------
            # one 512-wide proj chunk: psum <- W_mtile^T @ x_chunk
            def proj_mm(w_sb, mt, c):
                pt = ps_p.tile([P, 512], dt.float32, tag="p", name="pt")
                for k in range(KE):
                    lhsT = (w_sb[:, k, mt * P:(mt + 1) * P] if k < KO
                            else w_sb[0:1, k, mt * P:(mt + 1) * P])
                    rhs = (xTs[:, k, c * 512:(c + 1) * 512] if k < KO
                           else xTs[0:1, k, c * 512:(c + 1) * 512])
                    nc.tensor.matmul(pt[:], lhsT, rhs,
                                     start=(k == 0), stop=(k == KE - 1))
                return pt

            qk_stage = {}

            def qk_chunk(which, mt, c):
                # proj + fp8 cast; on odd c, fold-DMA the completed S-half
                # into the DoubleRow layout [32, h, j, S].
                w_sb = wqs if which == "q" else wks
                pt = proj_mm(w_sb, mt, c)
                if SCORES_FP8:
                    key = (which, mt, c // 2)
                    if c % 2 == 0:
                        qk_stage[key] = stg.tile([P, FB], dt.float8e4,
                                                 tag="stg", name="stg")
                    st = qk_stage[key]
                    nc.vector.tensor_copy(st[:, (c % 2) * 512:
                                             (c % 2) * 512 + 512], pt[:])
                    if c % 2 == 1:
                        dstt = qt8 if which == "q" else kt8
                        half = c // 2
                        for hip in range(2):
                            h = 2 * mt + hip
                            for j in range(2):
                                nc.sync.dma_start(
                                    dstt[0:32, h, j,
                                         half * FB:(half + 1) * FB],
                                    st[hip * 64 + 32 * j:
                                       hip * 64 + 32 * j + 32, :])
                        del qk_stage[key]
                else:
                    dstt = qtb if which == "q" else ktb
                    nc.vector.tensor_copy(
                        dstt[:, mt, c * 512:(c + 1) * 512], pt[:])

            vf_stage = {}

            def vf_chunk(mt, c):
                # feature-major V' proj chunk; after chunk 3, XBAR-transpose
                # both heads of this m-tile into token-major vp.
                pt = proj_mm(wvs, mt, c)
                if c == 0:
                    vf_stage[mt] = vfp.tile([P, S], dt.bfloat16,
                                            tag="vf", name="vf")
                vt = vf_stage[mt]
                nc.vector.tensor_copy(vt[:, c * 512:(c + 1) * 512], pt[:])
                if c == 3:
                    for hip in range(2):
                        h = 2 * mt + hip
                        # XBAR transpose needs a contiguous destination
                        # (strided dst slices produce wrong output on HW);
                        # bounce through a scratch tile, GpSimd copies into
                        # the strided vp layout.
                        vph = vfp.tile([P, NT, DH], dt.bfloat16,
                                       tag="vph", name="vph")
                        nc.sync.dma_start_transpose(
                            vph[:], vt[hip * DH:(hip + 1) * DH, :])
                        nc.vector.tensor_copy(
                            vp[:, :, h * P:h * P + DH], vph[:])
                    del vf_stage[mt]

            # ---------------- warmup (HAM un-throttle during DMA) --------
            # runs on uninitialized SBUF (results never read; psum slot is
            # reclaimed via start=True) so the PE starts at t~0 with no
            # upstream deps; psum from the proj pool so the scores pool
            # rotation keeps its full 2-slot lag.
            warm = sb.tile([P, 512], dt.bfloat16, tag="warm")
            nc.vector.memset(warm[:], 0.0)
            wexp = sb.tile([P, 1], dt.bfloat16, tag="wexp")
            nc.scalar.activation(wexp[:], warm[:, 0:1], Exp)
            ones1 = sb.tile([1, DH], dt.bfloat16, tag="ones1")
            nc.gpsimd.memset(ones1[:], 1.0)
            wpt = ps_p.tile([P, 512], dt.float32, tag="p", name="wpt")
            for wi in range(20):
                nc.tensor.matmul(wpt[:], warm[:, 0:P], warm[:],
                                 start=(wi == 0), stop=(wi == 19))

            # ---------------- prefix projections ----------------
            qk_chunk("q", 0, 0)
            qk_chunk("q", 0, 1)
            qk_chunk("k", 0, 0)
            qk_chunk("k", 0, 1)

            # ---------------- fill queue ----------------
            from collections import deque
            fills = deque()
            fills.append(lambda: vf_chunk(0, 0))
            fills.append(lambda: vf_chunk(0, 1))
            fills.append(lambda: vf_chunk(0, 2))
            fills.append(lambda: vf_chunk(0, 3))
            fills.append(lambda: qk_chunk("k", 0, 2))
            fills.append(lambda: qk_chunk("k", 0, 3))
            for mt in (1, 2):
                for c in range(4):
                    fills.append(lambda m=mt, cc=c: qk_chunk("q", m, cc))
                for c in range(4):
                    fills.append(lambda m=mt, cc=c: qk_chunk("k", m, cc))
                for c in range(4):
                    fills.append(lambda m=mt, cc=c: vf_chunk(m, cc))
            # q0c23 (needed by seg 2 = (h0,fb1)) goes right after seg0's
            # must-haves; m1/m2 chunks follow FIFO.
            fills.insert(6, lambda: qk_chunk("q", 0, 2))
            fills.insert(7, lambda: qk_chunk("q", 0, 3))

            def pop_fill(n):
                for _ in range(n):
                    if fills:
                        fills.popleft()()

            # ---------------- attention segments ----------------
            def emit_scores(h, fb, ti):
                s_ps = ps_s.tile([P, FB], dt.float32, tag="s", name="s_ps")
                if SCORES_FP8:
                    if TOK_INTERLEAVED:
                        ksel = kt8[0:32, h, :, :].rearrange(
                            "p j (t g) -> p j g t", g=NT)[:, :, ti, :]
                    else:
                        ksel = kt8[0:32, h, :, ti * P:(ti + 1) * P]
                    for n in range(2):
                        fc = fb * FB + n * 512
                        nc.tensor.matmul(
                            s_ps[:, n * 512:(n + 1) * 512], ksel,
                            qt8[0:32, h, :, fc:fc + 512],
                            start=True, stop=True, perf_mode=DR)
                else:
                    off = (h % 2) * DH
                    if TOK_INTERLEAVED:
                        ksel = ktb[off:off + DH, h // 2, :].rearrange(
                            "d (t g) -> d g t", g=NT)[:, ti, :]
                    else:
                        ksel = ktb[off:off + DH, h // 2,
                                   ti * P:(ti + 1) * P]
                    for n in range(2):
                        fc = fb * FB + n * 512
                        nc.tensor.matmul(
                            s_ps[:, n * 512:(n + 1) * 512], ksel,
                            qtb[off:off + DH, h // 2, fc:fc + 512],
                            start=True, stop=True)
                return s_ps

            def emit_exp(s_ps, ti):
                et = etp.tile([P, FB], dt.bfloat16, tag="et", name="et")
                bias = 0.0 if ones_mask else adder_sb[:, ti:ti + 1]
                nc.scalar.activation(et[:], s_ps[:], Exp,
                                     bias=bias, scale=0.125)
                return et

            def emit_ctx(ctx_ps, h, ti, et):
                for n in range(2):
                    nc.tensor.matmul(
                        ctx_ps[:, n * 512:(n + 1) * 512],
                        vp[:, ti, h * P:(h + 1) * P],
                        et[:, n * 512:(n + 1) * 512],
                        start=(ti == 0), stop=(ti == NT - 1))

            def normalize(h, fb, ctxs_t):
                nfb = FB // P   # 8 denominator cols
                den = fin.tile([P, nfb], dt.float32, tag="den", name="den")
                nc.sync.dma_start(den[:], ctxs_t[DH:DH + 1, :])
                rec = fin.tile([P, nfb], dt.float32, tag="rec", name="rec")
                nc.vector.reciprocal(rec[:], den[:])
                rr = fin.tile([1, FB], dt.float32, tag="rr", name="rr")
                nc.sync.dma_start(rr[:], rec[:])
                rrb = fin.tile([DH, FB], dt.float32, tag="rrb", name="rrb")
                nc.gpsimd.partition_broadcast(rrb[:], rr[:])
                ot = fin.tile([DH, FB], dt.float32, tag="ot", name="ot")
                nc.vector.tensor_tensor(ot[:], ctxs_t[0:DH, :], rrb[:],
                                        mybir.AluOpType.mult)
                nc.sync.dma_start(
                    out.ap()[h * DH:(h + 1) * DH, fb * FB:(fb + 1) * FB],
                    ot[:])

            segs = [(0, 0), (1, 0), (0, 1), (1, 1),
                    (2, 0), (3, 0), (2, 1), (3, 1),
                    (4, 0), (5, 0), (4, 1), (5, 1)]
            for si, (h, fb) in enumerate(segs):
                ctx_ps = ps_c.tile([P, FB], dt.float32, tag="c",
                                   name="ctx_ps")
                pend = deque()
                first = (si == 0)
                for ti in range(NT):
                    s_ps = emit_scores(h, fb, ti)
                    et = emit_exp(s_ps, ti)
                    pend.append((ti, et))
                    # ctx at lag>=1 (never same-slot: PE would stall on the
                    # exp); seg 0 defers until vp-h0 exists (slot 8+).
                    if first and ti < 8:
                        nctx = 0
                    else:
                        nctx = 2 if len(pend) > 2 else (
                            1 if len(pend) == 2 else 0)
                    for _ in range(nctx):
                        tj, etj = pend.popleft()
                        emit_ctx(ctx_ps, h, tj, etj)
                    # fill: ~6 proj chunks per segment, front-loaded in
                    # seg 0 (vf0+k0c23 must be emitted before ctx/scores
                    # that depend on them — PE executes in order).
                    if si == 0:
                        if ti < 6:
                            pop_fill(1)
                    elif ti % 3 == 1:
                        pop_fill(1)
                while pend:
                    tj, etj = pend.popleft()
                    emit_ctx(ctx_ps, h, tj, etj)
                # drain + normalize (off the PE critical path)
                ctxs_t = fin.tile([DH + 1, FB], dt.float32, tag="ctxs",
                                  name="ctxs")
                nc.vector.tensor_copy(ctxs_t[:], ctx_ps[0:DH + 1, :])
                if si < len(segs) - 1:
                    normalize(h, fb, ctxs_t)
                else:
                    # tail: minimize serial latency (no DMA gather/scatter,
                    # no gpsimd broadcast): DVE row-reciprocal + PE ones-
                    # broadcast into psum + DVE multiply, 512-chunked.
                    CW = 256
                    for n in range(FB // CW):
                        rrow = fin.tile([1, CW], dt.bfloat16, tag="rrow",
                                        name="rrow", bufs=4)
                        with nc.allow_low_precision(
                                reason="bf16 recip feeds PE broadcast; "
                                       "~0.2% rel err is in budget"):
                            nc.vector.reciprocal(
                                rrow[:],
                                ctxs_t[DH:DH + 1, n * CW:(n + 1) * CW])
                        rb = ps_s.tile([DH, CW], dt.float32, tag="s",
                                       name="rb")
                        nc.tensor.matmul(rb[:], ones1[:], rrow[:],
                                         start=True, stop=True)
                        ot = fin.tile([DH, CW], dt.float32, tag="ot2",
                                      name="ot2", bufs=4)
                        nc.vector.tensor_tensor(
                            ot[:], ctxs_t[0:DH, n * CW:(n + 1) * CW],
                            rb[:], mybir.AluOpType.mult)
                        nc.sync.dma_start(
                            out.ap()[h * DH:(h + 1) * DH,
                                     fb * FB + n * CW:
                                     fb * FB + (n + 1) * CW],
                            ot[:])
            while fills:
                fills.popleft()()

    nc.compile()
    return nc


def _prep_core_inputs(c, x, Wq, bq, Wk, bk, Wv, bv, mask, ones_mask):
    bf16 = ml_dtypes.bfloat16
    b, hg = c // 2, c % 2
    cols = slice(hg * DL, (hg + 1) * DL)

    xT_ = np.ascontiguousarray(x[b].T.astype(bf16))

    def aug(W, bias):
        w = np.empty((D + 1, DL), dtype=bf16)
        w[:D] = W[:, cols].astype(bf16)
        w[D] = bias[cols].astype(bf16)
        return w

    if ones_mask:
        adder_t = np.zeros((P, NT), dtype=np.float32)
    else:
        add = ((mask[b].astype(np.float32) - 1.0) * 10000.0)
        if TOK_INTERLEAVED:
            adder_t = add.reshape(P, NT).copy()      # [p, ti] = add[p*16+ti]
        else:
            adder_t = add.reshape(NT, P).T.copy()    # [p, ti] = add[ti*128+p]

    return {"xT": xT_, "wq": aug(Wq, bq), "wk": aug(Wk, bk),
            "wv": aug(Wv, bv),
            "adder": np.ascontiguousarray(adder_t, dtype=np.float32)}


def kernel(x, Wq, bq, Wk, bk, Wv, bv, mask, _trace=False):
    from concourse.bass_utils import run_bass_kernel_spmd

    x = np.asarray(x, dtype=np.float32)
    Wq = np.asarray(Wq, dtype=np.float32)
    bq = np.asarray(bq, dtype=np.float32)
    Wk = np.asarray(Wk, dtype=np.float32)
    bk = np.asarray(bk, dtype=np.float32)
    Wv = np.asarray(Wv, dtype=np.float32)
    bv = np.asarray(bv, dtype=np.float32)
    mask = np.asarray(mask)

    with_bias = bool(bq.any() or bk.any() or bv.any())
    ones_mask = bool((mask == 1).all())
    key = ("nc", with_bias, ones_mask)
    if key not in _CACHE:
        _CACHE[key] = _build(with_bias=with_bias, ones_mask=ones_mask)
    nc = _CACHE[key]

    in_maps = [_prep_core_inputs(c, x, Wq, bq, Wk, bk, Wv, bv, mask,
                                 ones_mask)
               for c in range(NCORES)]
    res = run_bass_kernel_spmd(nc, in_maps, core_ids=list(range(NCORES)),
                               trace=_trace)
    if _trace:
        _CACHE["last_result"] = res

    full = np.empty((B, S, D), dtype=np.float32)
    for c in range(NCORES):
        b, hg = c // 2, c % 2
        full[b, :, hg * DL:(hg + 1) * DL] = res.results[c]["out"].T
    return full


# revision 12
# speedup vs baseline: 1.5973x; 1.0436x over previous
"""Self-contained Trainium2 Bass kernel for a 12-head attention layer.

Problem: x[4,2048,768] -> attention(QKV projections, softmax, context),
NUM_HEADS=12, SIZE_PER_HEAD=64, additive mask from mask[4,2048].

Sharding over 8 NeuronCores: core c handles batch b=c//2 and head-group
hg=c%2 (6 heads, 384 feature columns).  Everything is local per core:
no collectives.

Design (v2):
  - Projections Q^T,K^T [384,2048] and V^T feature-major, all bf16 matmuls
    at full K=128/M=128 utilization; V^T is DMA-XBAR-transposed to a
    token-major vp [128, 16T, 6x(64+1)] layout (65th col = ones, memset)
    so the context matmul's 65th row yields the softmax denominator.
  - Scores: fp8(e4m3) DoubleRow matmuls (2 contraction values per PE
    cell: lhsT/rhs [32,2,*]) -> 2x PE throughput vs bf16 at K=64.  The
    1/sqrt(64) scale is folded into the exp activation's scale, so Q/K
    are quantized at natural scale.  Rel-err budget ~1.3% < 2e-2 gate.
  - Softmax: exp on ACT (the hard bottleneck: 192 x [128,1024] tiles
    ~206us); mask enters as per-partition bias (all-ones mask -> bias 0).
  - Single-head segments (6 heads x 2 F-blocks x 16 T-tiles), scores
    psum double-buffered, ctx psum single [65,1024] + copy-drain;
    projections interleaved as fill so ACT never starves.
  - Normalize per segment: DMA-gather denominators -> DVE reciprocal ->
    DMA scatter -> GpSimd partition_broadcast -> DVE multiply -> DMA out.

Output per core: ctx^T [384,2048] f32; host transposes to [4,2048,768].
"""

import numpy as np
import ml_dtypes

B, S, D = 4, 2048, 768
H, DH = 12, 64
HL = 6          # heads per core
DL = HL * DH    # 384 feature columns per core
NCORES = 8
P = 128
KO = 6          # full k-subtiles of the 768 contraction
NT = S // P     # 16 T-tiles
FB = 1024       # F block size
NFB = S // FB   # 2 F blocks

SCORES_FP8 = False
# token layout inside a T-tile, set by the V transpose DMA semantics:
# True: vp[p, mt, :] holds token p*NT + mt  (XBAR row-major order)
# False: vp[p, mt, :] holds token mt*P + p  (natural; probe-verified)
TOK_INTERLEAVED = False

FP8 = ml_dtypes.float8_e4m3

_CACHE = {}


def _build(with_bias=False, ones_mask=True):
    import concourse.mybir as mybir
    import concourse.tile as tile
    from concourse import bacc

    dt = mybir.dt
    Exp = mybir.ActivationFunctionType.Exp
    DR = mybir.MatmulPerfMode.DoubleRow

    nc = bacc.Bacc("TRN2", target_bir_lowering=False, debug=False,
                   num_devices=NCORES)

    xT = nc.dram_tensor("xT", [D, S], dt.bfloat16, kind="ExternalInput")
    wq = nc.dram_tensor("wq", [D + 1, DL], dt.bfloat16, kind="ExternalInput")
    wk = nc.dram_tensor("wk", [D + 1, DL], dt.bfloat16, kind="ExternalInput")
    wv = nc.dram_tensor("wv", [D + 1, DL], dt.bfloat16, kind="ExternalInput")
    adder = nc.dram_tensor("adder", [P, NT], dt.float32, kind="ExternalInput")
    out = nc.dram_tensor("out", [DL, S], dt.float32, kind="ExternalOutput")

    KE = KO + 1 if with_bias else KO

    with tile.TileContext(nc) as tc:
        with (
            tc.tile_pool(name="persist", bufs=1) as sb,
            tc.tile_pool(name="etp", bufs=10) as etp,
            tc.tile_pool(name="stage", bufs=3) as stg,
            tc.tile_pool(name="vfp", bufs=2) as vfp,
            tc.tile_pool(name="fin", bufs=2) as fin,
            tc.tile_pool(name="ps_s", bufs=2, space="PSUM") as ps_s,
            tc.tile_pool(name="ps_c", bufs=1, space="PSUM") as ps_c,
            tc.tile_pool(name="ps_p", bufs=2, space="PSUM") as ps_p,
        ):
            # ---------------- input DMA (priority order) ----------------
            wqs = sb.tile([P, KO + 1, DL], dt.bfloat16, tag="wqs")
            wks = sb.tile([P, KO + 1, DL], dt.bfloat16, tag="wks")
            wvs = sb.tile([P, KO + 1, DL], dt.bfloat16, tag="wvs")
            xTs = sb.tile([P, KO + 1, S], dt.bfloat16, tag="xTs")
            adder_sb = sb.tile([P, NT], dt.float32, tag="adder")

            def dma_w(w_dram, w_sb, c0, c1):
                nc.sync.dma_start(
                    w_sb[:, 0:KO, c0:c1],
                    w_dram.ap()[0:D, c0:c1].rearrange(
                        "(ko p) m -> p ko m", p=P))
                if with_bias:
                    nc.sync.dma_start(w_sb[0:1, KO, c0:c1],
                                      w_dram.ap()[D:D + 1, c0:c1])

            def dma_x(s0, s1):
                nc.sync.dma_start(
                    xTs[:, 0:KO, s0:s1],
                    xT.ap()[:, s0:s1].rearrange("(ko p) s -> p ko s", p=P))

            dma_w(wq, wqs, 0, P)
            dma_w(wk, wks, 0, P)
            dma_x(0, FB)
            dma_w(wv, wvs, 0, DL)
            dma_x(FB, S)
            dma_w(wq, wqs, P, DL)
            dma_w(wk, wks, P, DL)
            if not ones_mask:
                nc.sync.dma_start(adder_sb[:], adder.ap())
            if with_bias:
                nc.gpsimd.memset(xTs[0:1, KO, :], 1.0)

            # persistent projection outputs
            if SCORES_FP8:
                qt8 = sb.tile([32, HL, 2, S], dt.float8e4, tag="qt8")
                kt8 = sb.tile([32, HL, 2, S], dt.float8e4, tag="kt8")
            else:
                qtb = sb.tile([P, 3, S], dt.bfloat16, tag="qtb")
                ktb = sb.tile([P, 3, S], dt.bfloat16, tag="ktb")
            vp = sb.tile([P, NT, HL * P], dt.bfloat16, tag="vp")
            for h in range(HL):
                nc.gpsimd.memset(vp[:, :, h * P + DH:h * P + DH + 1], 1.0)
                nc.gpsimd.memset(vp[:, :, h * P + DH + 1:(h + 1) * P], 0.0)

            # ---------------- projection helpers ----------------
            # one 512-wide proj chunk: psum <- W_mtile^T @ x_chunk
            def proj_mm(w_sb, mt, c):
                pt = ps_p.tile([P, 512], dt.float32, tag="p", name="pt")
                for k in range(KE):
                    lhsT = (w_sb[:, k, mt * P:(mt + 1) * P] if k < KO
                            else w_sb[0:1, k, mt * P:(mt + 1) * P])
                    rhs = (xTs[:, k, c * 512:(c + 1) * 512] if k < KO
                           else xTs[0:1, k, c * 512:(c + 1) * 512])
                    nc.tensor.matmul(pt[:], lhsT, rhs,
                                     start=(k == 0), stop=(k == KE - 1))
                return pt

            qk_stage = {}

            def qk_chunk(which, mt, c):
                # proj + fp8 cast; on odd c, fold-DMA the completed S-half
                # into the DoubleRow layout [32, h, j, S].
                w_sb = wqs if which == "q" else wks
                pt = proj_mm(w_sb, mt, c)
                if SCORES_FP8:
                    key = (which, mt, c // 2)
                    if c % 2 == 0:
                        qk_stage[key] = stg.tile([P, FB], dt.float8e4,
                                                 tag="stg", name="stg")
                    st = qk_stage[key]
                    nc.vector.tensor_copy(st[:, (c % 2) * 512:
                                             (c % 2) * 512 + 512], pt[:])
                    if c % 2 == 1:
                        dstt = qt8 if which == "q" else kt8
                        half = c // 2
                        for hip in range(2):
                            h = 2 * mt + hip
                            for j in range(2):
                                nc.sync.dma_start(
                                    dstt[0:32, h, j,
                                         half * FB:(half + 1) * FB],
                                    st[hip * 64 + 32 * j:
                                       hip * 64 + 32 * j + 32, :])
                        del qk_stage[key]
                else:
                    dstt = qtb if which == "q" else ktb
                    nc.vector.tensor_copy(
                        dstt[:, mt, c * 512:(c + 1) * 512], pt[:])

            vf_stage = {}

            def vf_chunk(mt, c):
                # feature-major V' proj chunk; after chunk 3, XBAR-transpose
                # both heads of this m-tile into token-major vp.
                pt = proj_mm(wvs, mt, c)
                if c == 0:
                    vf_stage[mt] = vfp.tile([P, S], dt.bfloat16,
                                            tag="vf", name="vf")
                vt = vf_stage[mt]
                nc.vector.tensor_copy(vt[:, c * 512:(c + 1) * 512], pt[:])
                if c == 3:
                    for hip in range(2):
                        h = 2 * mt + hip
                        # XBAR transpose needs a contiguous destination
                        # (strided dst slices produce wrong output on HW);
                        # bounce through a scratch tile, GpSimd copies into
                        # the strided vp layout.
                        vph = vfp.tile([P, NT, DH], dt.bfloat16,
                                       tag="vph", name="vph")
                        nc.sync.dma_start_transpose(
                            vph[:], vt[hip * DH:(hip + 1) * DH, :])
                        nc.vector.tensor_copy(
                            vp[:, :, h * P:h * P + DH], vph[:])
                    del vf_stage[mt]

            # ---------------- warmup (HAM un-throttle during DMA) --------
            # runs on uninitialized SBUF (results never read; psum slot is
            # reclaimed via start=True) so the PE starts at t~0 with no
            # upstream deps; psum from the proj pool so the scores pool
            # rotation keeps its full 2-slot lag.
            warm = sb.tile([P, 512], dt.bfloat16, tag="warm")
            nc.vector.memset(warm[:], 0.0)
            wexp = sb.tile([P, 1], dt.bfloat16, tag="wexp")
            nc.scalar.activation(wexp[:], warm[:, 0:1], Exp)
            ones1 = sb.tile([1, DH], dt.bfloat16, tag="ones1")
            nc.gpsimd.memset(ones1[:], 1.0)
            wpt = ps_p.tile([P, 512], dt.float32, tag="p", name="wpt")
            for wi in range(20):
                nc.tensor.matmul(wpt[:], warm[:, 0:P], warm[:],
                                 start=(wi == 0), stop=(wi == 19))

            # ---------------- prefix projections ----------------
            qk_chunk("q", 0, 0)
            qk_chunk("q", 0, 1)
            qk_chunk("k", 0, 0)
            qk_chunk("k", 0, 1)

            # ---------------- fill queue ----------------
            from collections import deque
            fills = deque()
            fills.append(lambda: vf_chunk(0, 0))
            fills.append(lambda: vf_chunk(0, 1))
            fills.append(lambda: vf_chunk(0, 2))
            fills.append(lambda: vf_chunk(0, 3))
            fills.append(lambda: qk_chunk("k", 0, 2))
            fills.append(lambda: qk_chunk("k", 0, 3))
            for mt in (1, 2):
                for c in range(4):
                    fills.append(lambda m=mt, cc=c: qk_chunk("q", m, cc))
                for c in range(4):
                    fills.append(lambda m=mt, cc=c: qk_chunk("k", m, cc))
                for c in range(4):
                    fills.append(lambda m=mt, cc=c: vf_chunk(m, cc))
            # q0c23 (needed by seg 2 = (h0,fb1)) goes right after seg0's
            # must-haves; m1/m2 chunks follow FIFO.
            fills.insert(6, lambda: qk_chunk("q", 0, 2))
            fills.insert(7, lambda: qk_chunk("q", 0, 3))

            def pop_fill(n):
                for _ in range(n):
                    if fills:
                        fills.popleft()()

            # ---------------- attention segments ----------------
            def emit_scores(h, fb, ti):
                s_ps = ps_s.tile([P, FB], dt.float32, tag="s", name="s_ps")
                if SCORES_FP8:
                    if TOK_INTERLEAVED:
                        ksel = kt8[0:32, h, :, :].rearrange(
                            "p j (t g) -> p j g t", g=NT)[:, :, ti, :]
                    else:
                        ksel = kt8[0:32, h, :, ti * P:(ti + 1) * P]
                    for n in range(2):
                        fc = fb * FB + n * 512
                        nc.tensor.matmul(
                            s_ps[:, n * 512:(n + 1) * 512], ksel,
                            qt8[0:32, h, :, fc:fc + 512],
                            start=True, stop=True, perf_mode=DR)
                else:
                    off = (h % 2) * DH
                    if TOK_INTERLEAVED:
                        ksel = ktb[off:off + DH, h // 2, :].rearrange(
                            "d (t g) -> d g t", g=NT)[:, ti, :]
                    else:
                        ksel = ktb[off:off + DH, h // 2,
                                   ti * P:(ti + 1) * P]
                    for n in range(2):
                        fc = fb * FB + n * 512
                        nc.tensor.matmul(
                            s_ps[:, n * 512:(n + 1) * 512], ksel,
                            qtb[off:off + DH, h // 2, fc:fc + 512],
                            start=True, stop=True)
                return s_ps

            def emit_exp(s_ps, ti):
                et = etp.tile([P, FB], dt.bfloat16, tag="et", name="et")
                bias = 0.0 if ones_mask else adder_sb[:, ti:ti + 1]
                nc.scalar.activation(et[:], s_ps[:], Exp,
                                     bias=bias, scale=0.125)
                return et

            def emit_ctx(ctx_ps, h, ti, et):
                for n in range(2):
                    nc.tensor.matmul(
                        ctx_ps[:, n * 512:(n + 1) * 512],
                        vp[:, ti, h * P:(h + 1) * P],
                        et[:, n * 512:(n + 1) * 512],
                        start=(ti == 0), stop=(ti == NT - 1))

            def normalize(h, fb, ctxs_t):
                nfb = FB // P   # 8 denominator cols
                den = fin.tile([P, nfb], dt.float32, tag="den", name="den")
                nc.sync.dma_start(den[:], ctxs_t[DH:DH + 1, :])
                rec = fin.tile([P, nfb], dt.float32, tag="rec", name="rec")
                nc.vector.reciprocal(rec[:], den[:])
                rr = fin.tile([1, FB], dt.float32, tag="rr", name="rr")
                nc.sync.dma_start(rr[:], rec[:])
                rrb = fin.tile([DH, FB], dt.float32, tag="rrb", name="rrb")
                nc.gpsimd.partition_broadcast(rrb[:], rr[:])
                ot = fin.tile([DH, FB], dt.float32, tag="ot", name="ot")
                nc.vector.tensor_tensor(ot[:], ctxs_t[0:DH, :], rrb[:],
                                        mybir.AluOpType.mult)
                nc.sync.dma_start(
                    out.ap()[h * DH:(h + 1) * DH, fb * FB:(fb + 1) * FB],
                    ot[:])

            segs = [(0, 0), (1, 0), (0, 1), (1, 1),
                    (2, 0), (3, 0), (2, 1), (3, 1),
                    (4, 0), (5, 0), (4, 1), (5, 1)]
            for si, (h, fb) in enumerate(segs):
                ctx_ps = ps_c.tile([P, FB], dt.float32, tag="c",
                                   name="ctx_ps")
                pend = deque()
                first = (si == 0)
                for ti in range(NT):
                    s_ps = emit_scores(h, fb, ti)
                    et = emit_exp(s_ps, ti)
                    pend.append((ti, et))
                    # ctx at lag>=1 (never same-slot: PE would stall on the
                    # exp); seg 0 defers until vp-h0 exists (slot 8+).
                    if first and ti < 8:
                        nctx = 0
                    else:
                        nctx = 2 if len(pend) > 3 else (
                            1 if len(pend) == 3 else 0)
                    for _ in range(nctx):
                        tj, etj = pend.popleft()
                        emit_ctx(ctx_ps, h, tj, etj)
                    # fill: ~6 proj chunks per segment, front-loaded in
                    # seg 0 (vf0+k0c23 must be emitted before ctx/scores
                    # that depend on them — PE executes in order).
                    if si == 0:
                        if ti < 6:
                            pop_fill(1)
                    elif ti % 3 == 1:
                        pop_fill(1)
                while pend:
                    tj, etj = pend.popleft()
                    emit_ctx(ctx_ps, h, tj, etj)
                # drain + normalize (off the PE critical path)
                ctxs_t = fin.tile([DH + 1, FB], dt.float32, tag="ctxs",
                                  name="ctxs")
                nc.vector.tensor_copy(ctxs_t[:], ctx_ps[0:DH + 1, :])
                if si < len(segs) - 1:
                    normalize(h, fb, ctxs_t)
                else:
                    # tail: minimize serial latency (no DMA gather/scatter,
                    # no gpsimd broadcast): DVE row-reciprocal + PE ones-
                    # broadcast into psum + DVE multiply, 512-chunked.
                    den = fin.tile([P, FB // P], dt.float32,
                                   tag="den", name="den")
                    nc.sync.dma_start(den[:], ctxs_t[DH:DH + 1, :])
                    rec = fin.tile([P, FB // P], dt.bfloat16, tag="rec",
                                   name="rec")
                    with nc.allow_low_precision(
                            reason="bf16 recip feeds PE broadcast; "
                                   "~0.2% rel err is in budget"):
                        nc.vector.reciprocal(rec[:], den[:])
                    rrow = fin.tile([1, FB], dt.bfloat16, tag="rrow",
                                    name="rrow")
                    nc.sync.dma_start(rrow[:], rec[:])
                    for n in range(2):
                        rb = ps_s.tile([DH, 512], dt.float32, tag="s",
                                       name="rb")
                        nc.tensor.matmul(rb[:], ones1[:],
                                         rrow[:, n * 512:(n + 1) * 512],
                                         start=True, stop=True)
                        ot = fin.tile([DH, 512], dt.float32, tag="ot2",
                                      name="ot2", bufs=4)
                        nc.vector.tensor_tensor(
                            ot[:], ctxs_t[0:DH, n * 512:(n + 1) * 512],
                            rb[:], mybir.AluOpType.mult)
                        nc.sync.dma_start(
                            out.ap()[h * DH:(h + 1) * DH,
                                     fb * FB + n * 512:
                                     fb * FB + (n + 1) * 512],
                            ot[:])
            while fills:
                fills.popleft()()

    nc.compile()
    return nc


def _prep_core_inputs(c, x, Wq, bq, Wk, bk, Wv, bv, mask, ones_mask):
    bf16 = ml_dtypes.bfloat16
    b, hg = c // 2, c % 2
    cols = slice(hg * DL, (hg + 1) * DL)

    xT_ = np.ascontiguousarray(x[b].T.astype(bf16))

    def aug(W, bias):
        w = np.empty((D + 1, DL), dtype=bf16)
        w[:D] = W[:, cols].astype(bf16)
        w[D] = bias[cols].astype(bf16)
        return w

    if ones_mask:
        adder_t = np.zeros((P, NT), dtype=np.float32)
    else:
        add = ((mask[b].astype(np.float32) - 1.0) * 10000.0)
        if TOK_INTERLEAVED:
            adder_t = add.reshape(P, NT).copy()      # [p, ti] = add[p*16+ti]
        else:
            adder_t = add.reshape(NT, P).T.copy()    # [p, ti] = add[ti*128+p]

    return {"xT": xT_, "wq": aug(Wq, bq), "wk": aug(Wk, bk),
            "wv": aug(Wv, bv),
            "adder": np.ascontiguousarray(adder_t, dtype=np.float32)}


def kernel(x, Wq, bq, Wk, bk, Wv, bv, mask, _trace=False):
    from concourse.bass_utils import run_bass_kernel_spmd

    x = np.asarray(x, dtype=np.float32)
    Wq = np.asarray(Wq, dtype=np.float32)
    bq = np.asarray(bq, dtype=np.float32)
    Wk = np.asarray(Wk, dtype=np.float32)
    bk = np.asarray(bk, dtype=np.float32)
    Wv = np.asarray(Wv, dtype=np.float32)
    bv = np.asarray(bv, dtype=np.float32)
    mask = np.asarray(mask)

    with_bias = bool(bq.any() or bk.any() or bv.any())
    ones_mask = bool((mask == 1).all())
    key = ("nc", with_bias, ones_mask)
    if key not in _CACHE:
        _CACHE[key] = _build(with_bias=with_bias, ones_mask=ones_mask)
    nc = _CACHE[key]

    in_maps = [_prep_core_inputs(c, x, Wq, bq, Wk, bk, Wv, bv, mask,
                                 ones_mask)
               for c in range(NCORES)]
    res = run_bass_kernel_spmd(nc, in_maps, core_ids=list(range(NCORES)),
                               trace=_trace)
    if _trace:
        _CACHE["last_result"] = res

    full = np.empty((B, S, D), dtype=np.float32)
    for c in range(NCORES):
        b, hg = c // 2, c % 2
        full[b, :, hg * DL:(hg + 1) * DL] = res.results[c]["out"].T
    return full
